# revision 8
# baseline (speedup 1.0000x reference)
"""TRN2 Bass kernel for nn_Attender: weights[b,s] = sum_d (state@W.T+b)[b,d] * enc[s,b,d].

Sharding: the contraction dim D (=2048) is split into 8 slices of 256, one per
NeuronCore. Each core computes altered[:, d_k] = state @ W[d_k, :].T + bias[d_k]
(needs only a 256-row slice of W) and the partial score
partial_k[b, s] = sum_{d in d_k} altered[b, d] * enc[s, b, d].
The host sums the 8 partials — no cross-device communication.

Device layout (host-prepared, all contiguous DMA):
  enc   [2, 128, 4, F]  per-core enc slice, transposed to put d on partitions:
                        [chunk c, partition p, batch-group g, free]
                        free = (hi/lo, b_in_group, s) for bf16x3, (b_in_group, s) for f32r
  wp    [128, 4096]     wp[p, i*256+d] = W[k*256+d, i*128+p]   (lhsT tiles for altered)
  sp    [128, 256]      sp[p, i*16+b]  = state[b, i*128+p]
  bk    [128, 2]        bk[p, c]       = bias[k*256 + c*128 + p]

Main contraction on the PE: lhsT = alteredT[d_chunk, b] (M=1), rhs = encT[d_chunk, s]
(N=512), accumulated over chunks (and hi/lo passes) in PSUM.

Precision modes:
  bf16x3: enc and altered split into bf16 hi+lo on host/device; 3 accumulated
          products (hi*hi + lo*hi + hi*lo) -> ~1e-5 scale-relative error.
  f32r:   single float32r pass -> ~6e-4 scale-relative error, 3x fewer matmuls.
"""

import os
from contextlib import ExitStack

import numpy as np
import ml_dtypes

import concourse.bacc as bacc
import concourse.tile as tile
import concourse.mybir as mybir
from concourse.bass_utils import run_bass_kernel_spmd

S, B, D = 2048, 16, 2048
NCORES = 8
DK = D // NCORES  # 256 contraction elems per core
NCH = DK // 128  # 2 partition chunks
BG = 4  # batches per group
NG = B // BG  # 4 groups
ST = 512  # s-tile (one PSUM bank)
NST = S // ST  # 4 s-tiles

MODE = os.environ.get("BASS_KERNEL_MODE", "bf16x3")

F32 = mybir.dt.float32
F32R = mybir.dt.float32r
BF16 = mybir.dt.bfloat16

_CACHE = {}

LAST_RESULTS = None


def _build(mode):
    nc = bacc.Bacc("TRN2", target_bir_lowering=False, debug=False, num_devices=NCORES)

    if mode == "bf16x3":
        efree = 2 * S
        edt = BF16
    else:
        efree = S
        edt = F32R
    ENC = nc.dram_tensor("enc", [NCH, 128, B, efree], edt, kind="ExternalInput").ap()
    WP = nc.dram_tensor("wp", [128, 16 * DK], F32, kind="ExternalInput").ap()
    SP = nc.dram_tensor("sp", [128, 16 * B], F32, kind="ExternalInput").ap()
    BK = nc.dram_tensor("bk", [128, NCH], F32, kind="ExternalInput").ap()
    OUT = nc.dram_tensor("out", [B, S], F32, kind="ExternalOutput").ap()

    with tile.TileContext(nc) as tc, ExitStack() as ctx:
        cpool = ctx.enter_context(tc.tile_pool(name="const", bufs=1))
        epool = ctx.enter_context(tc.tile_pool(name="enc", bufs=4))
        opool = ctx.enter_context(tc.tile_pool(name="outp", bufs=2))
        apsum = ctx.enter_context(tc.tile_pool(name="apsum", bufs=2, space="PSUM"))
        mpsum = ctx.enter_context(tc.tile_pool(name="mpsum", bufs=6, space="PSUM"))

        # Constants ride the SWDGE (gpsimd) path so the HWDGE (sync) queue
        # streams enc tiles from instruction 0.
        wp_t = cpool.tile([128, 16 * DK], F32, tag="wp")
        nc.gpsimd.dma_start(wp_t[:], WP[:])
        sp_t = cpool.tile([128, 16 * B], F32, tag="sp")
        nc.gpsimd.dma_start(sp_t[:], SP[:])
        bk_t = cpool.tile([128, NCH], F32, tag="bk")
        nc.gpsimd.dma_start(bk_t[:], BK[:])

        # alteredT[d, b] = sum_i W[d, i] * state[b, i] + bias[d], d on partitions.
        amats = []  # amats[c] = list of lhsT tiles for the passes
        for c in range(NCH):
            aps = apsum.tile([128, B], F32, tag="aps")
            for i in range(16):
                nc.tensor.matmul(
                    aps[:],
                    wp_t[:, i * DK + c * 128 : i * DK + (c + 1) * 128],
                    sp_t[:, i * B : (i + 1) * B],
                    start=(i == 0),
                    stop=(i == 15),
                )
            altf = cpool.tile([128, B], F32, tag=f"altf{c}")
            nc.vector.tensor_scalar_add(altf[:], aps[:], bk_t[:, c : c + 1])
            if mode == "bf16x3":
                ahi = cpool.tile([128, B], BF16, tag=f"ahi{c}")
                nc.vector.tensor_copy(ahi[:], altf[:])
                ahif = cpool.tile([128, B], F32, tag=f"ahif{c}")
                nc.vector.tensor_copy(ahif[:], ahi[:])
                alof = cpool.tile([128, B], F32, tag=f"alof{c}")
                nc.vector.tensor_sub(alof[:], altf[:], ahif[:])
                alo = cpool.tile([128, B], BF16, tag=f"alo{c}")
                nc.vector.tensor_copy(alo[:], alof[:])
                amats.append([ahi, alo])
            else:
                ar = cpool.tile([128, B], F32R, tag=f"ar{c}")
                nc.vector.tensor_copy(ar[:], altf[:])
                amats.append([ar])

        # passes: (a-tile index, enc hi/lo index)
        if mode == "bf16x3":
            passes = [(0, 0), (1, 0), (0, 1)]
        else:
            passes = [(0, 0)]
        n_mm = len(passes) * NCH

        out_r = OUT.rearrange("(g bi) s -> g bi s", bi=BG)
        for g in range(NG):
            pts = [
                mpsum.tile([128, ST], F32, tag="mm", name=f"pt_{g}_{st}")
                for st in range(NST)
            ]
            for bi in range(BG):
                b = g * BG + bi
                etiles = []
                for c in range(NCH):
                    et = epool.tile([128, efree], edt, tag=f"enc{c}")
                    nc.sync.dma_start(et[:], ENC[c, :, b, :])
                    etiles.append(et)
                for st in range(NST):
                    k = 0
                    for aj, hl in passes:
                        for c in range(NCH):
                            off = hl * S + st * ST
                            nc.tensor.matmul(
                                pts[st][32 * bi : 32 * bi + 1, :],
                                amats[c][aj][:, b : b + 1],
                                etiles[c][:, off : off + ST],
                                start=(k == 0),
                                stop=(k == n_mm - 1),
                                tile_position=(0, 32 * bi),
                            )
                            k += 1
            # Stage the group's [4, S] result (batch bi at partition 32*bi)
            # and ship it while later groups stream.
            outg = opool.tile([128, S], F32, tag="outg")
            for st in range(NST):
                dst = outg[:, st * ST : (st + 1) * ST]
                if st % 2 == 0:
                    nc.vector.tensor_copy(dst, pts[st][:])
                else:
                    nc.scalar.copy(dst, pts[st][:])
            src_r = outg[:].rearrange("(bi r) s -> bi r s", bi=BG)[:, 0]
            nc.sync.dma_start(out_r[g], src_r)

    nc.compile()
    return nc


def _prep_inputs(encoder_outputs, state, W, b, mode):
    """Build the 8 per-core input maps (heavy layout work on host)."""
    f32 = np.float32
    bf16 = ml_dtypes.bfloat16
    in_maps = []
    # [S, B, D] -> [B, D, S] once
    encT = np.ascontiguousarray(encoder_outputs.transpose(1, 2, 0))
    for k in range(NCORES):
        d0 = k * DK
        e = encT[:, d0 : d0 + DK, :]  # [B, DK, S]
        # -> [c, p, B, S]
        e = np.ascontiguousarray(e.reshape(B, NCH, 128, S).transpose(1, 2, 0, 3))
        if mode == "bf16x3":
            ehi = e.astype(bf16)
            elo = (e - ehi.astype(f32)).astype(bf16)
            # [c, p, B, S] -> [c, p, b, hl, s] -> [c, p, b, 2S]
            enc_k = np.stack([ehi, elo], axis=3).reshape(NCH, 128, B, 2 * S)
            enc_k = np.ascontiguousarray(enc_k)
        else:
            enc_k = np.ascontiguousarray(e)
        wp = np.ascontiguousarray(
            W[d0 : d0 + DK, :].T.reshape(16, 128, DK).transpose(1, 0, 2).reshape(128, 16 * DK)
        )
        sp = np.ascontiguousarray(
            state.T.reshape(16, 128, B).transpose(1, 0, 2).reshape(128, 16 * B)
        )
        bk = np.ascontiguousarray(b[d0 : d0 + DK].reshape(NCH, 128).T)
        in_maps.append({"enc": enc_k, "wp": wp, "sp": sp, "bk": bk})
    return in_maps


def kernel(encoder_outputs, state, W, b):
    global LAST_RESULTS
    mode = MODE
    if mode not in _CACHE:
        _CACHE[mode] = _build(mode)
    nc = _CACHE[mode]
    in_maps = _prep_inputs(
        np.asarray(encoder_outputs, dtype=np.float32),
        np.asarray(state, dtype=np.float32),
        np.asarray(W, dtype=np.float32),
        np.asarray(b, dtype=np.float32),
        mode,
    )
    res = run_bass_kernel_spmd(nc, in_maps, core_ids=list(range(NCORES)))
    LAST_RESULTS = res
    acc = np.zeros((B, S), dtype=np.float64)
    for k in range(NCORES):
        acc += res.results[k]["out"].astype(np.float64)
    return acc.astype(np.float32)
